# revision 37
# baseline (speedup 1.0000x reference)
"""Trainium2 Bass kernel for nn_ItemVectorTransform.

out = concat([x, softmax(x @ M.T) @ M], -1)   x:[2048,50] f32, M:[100000,50] f32

V2 strategy: K-sharded over 8 cores (12500 rows of M each, padded to
12544 = 98 chunks of 128), full batch per core. Host sums the 8 partial
(numerator, denominator) accumulators and divides.

Per core, chunks are processed in PAIRS to exploit PE array tiling:
  MM1 (scores, contraction D=50 <= 64): even chunk in PE rows 0-63,
      odd chunk in rows 64-127 -> two concurrent row-tiled matmuls.
  MM2 (readout, out partitions 51 <= 64): even chunk accumulates into
      PSUM partitions 0-50, odd into 64-114 -> two concurrent col-tiled
      matmuls.  Host adds the even/odd partials.

exp is the throughput floor (B*K/8 = 25.7M elems/core). It is split
between the Scalar engine (true exp, bf16 out) and the Vector engine
using the Schraudolph bit-trick: y = s*c1 + c2 + 1.5*2^23 in f32; the
low 16 bits of y's bit pattern are a bf16 approximating exp(s-25)
(~3% max rel err on those chunks only; softmax normalization cancels
most of it; measured end-to-end rel err ~3e-3 at 1/3 DVE share).

Data layouts (host prep):
  xt   [50, 2048]  fp16  x^T, DMA'd into SBUF partitions 0-49 AND 64-113
  mtp  [128, 49*128] fp16  M^T chunk pairs: even chunk d-rows at
                           partitions 0-49, odd at 64-113
  mnp  [128, 98*51] bf16   [M|1] with k on partitions, chunk-major cols
"""

import os
import sys

for _p in ("/opt/trn_rl_repo", "/root/.axon_site/_ro/trn_rl_repo"):
    if os.path.isdir(_p) and _p not in sys.path:
        sys.path.insert(0, _p)

import numpy as np
import ml_dtypes

import concourse.bacc as bacc
import concourse.mybir as mybir
from concourse import tile
from concourse.bass_utils import run_bass_kernel_spmd

B, K, D = 2048, 100000, 50
N_CORES = 8
KSH = K // N_CORES         # 12500 rows per core
NCHUNK = 98                # chunks of 128 (12544 padded)
KSHP = NCHUNK * 128
NPAIR = NCHUNK // 2        # 49
NBT = 4                    # batch tiles
BT = B // NBT              # 512
DP1 = D + 1                # 51
EXP_BIAS = -25.0
LOG2E = 1.4426950408889634

# Schraudolph-on-DVE constants: y = s*C1 + C2; low 16 bits of f32(y) are
# the bf16 pattern of ~exp(s + EXP_BIAS).  MAGIC forces round-to-int.
SCH_CORR = 0.043036
MAGIC = 12582912.0         # 1.5 * 2^23
C1 = 128.0 * LOG2E
C2 = 128.0 * (127.0 + EXP_BIAS * LOG2E - SCH_CORR) + MAGIC

DVE_PERIOD = 2             # every DVE_PERIOD-th unit runs exp on DVE
DVE_SLOT = 1
LAG = 4                    # units of skew between exp and MM2 readout
NBT_LIVE = 2               # b-tiles accumulated per sweep (PSUM: 3 st + 2 acc)
NWARM = 12                 # dummy matmuls to warm the PE HAM clock gate
PAIR_GROUPS = [2, 5, 10, 16, 16]  # DMA group sizes (pairs), sum = 49

_nc_cache = None


def _install_trace_support():
    """The container's antenv lacks axon_hooks; synthesize it from trn_boot's
    ctypes NTFF shim so run_bass_kernel_spmd(trace=True) can profile."""
    import types

    if "antenv.axon_hooks" not in sys.modules:
        bootdir = "/root/.axon_site/trn_agent_boot"
        if bootdir not in sys.path:
            sys.path.insert(0, bootdir)
        import trn_boot

        hook = trn_boot._ntff_profile_via_ctypes("/opt/axon/libaxon_pjrt.so")
        mod = types.ModuleType("antenv.axon_hooks")
        mod.get_axon_ntff_profile_hook = lambda: hook
        mod.set_axon_ntff_profile_hook = lambda h: None
        sys.modules["antenv.axon_hooks"] = mod

    import concourse.bass_utils as bu

    bu.upload_artifacts = lambda tmpdir: tmpdir


def _group_of_pair(p):
    p0 = 0
    for gi, n in enumerate(PAIR_GROUPS):
        if p < p0 + n:
            return gi, p0
        p0 += n
    raise ValueError(p)


def _build():
    fp16 = mybir.dt.float16
    bf16 = mybir.dt.bfloat16
    f32 = mybir.dt.float32
    Exp = mybir.ActivationFunctionType.Exp
    mult = mybir.AluOpType.mult
    add = mybir.AluOpType.add

    nc = bacc.Bacc("TRN2", debug=False, num_devices=N_CORES)
    xt_d = nc.dram_tensor("xt", [D, B], fp16, kind="ExternalInput")
    mtp_d = nc.dram_tensor("mtp", [128, NPAIR * 128], fp16, kind="ExternalInput")
    mnp_d = nc.dram_tensor("mnp", [128, NCHUNK * DP1], bf16, kind="ExternalInput")
    out_d = nc.dram_tensor("outU", [128, B], bf16, kind="ExternalOutput")

    with tile.TileContext(nc) as tc:
        with (
            tc.tile_pool(name="const", bufs=1) as constp,
            tc.tile_pool(name="sts", bufs=3, space="PSUM") as st_pool,
            tc.tile_pool(name="accp", bufs=1, space="PSUM") as acc_pool,
            tc.tile_pool(name="pts", bufs=8) as pt_pool,
        ):
            xtd = constp.tile([128, B], fp16)
            bias = constp.tile([128, 1], f32)
            nc.vector.memset(bias[:], EXP_BIAS)

            # DMA issue order tuned so the first compute gate (group 0 +
            # xtd cols 0:512) lands as early as possible.
            mt_g, mn_g = [], []
            p0 = 0
            for gi, ng in enumerate(PAIR_GROUPS):
                mtg = constp.tile(
                    [128, ng * 128], fp16, name=f"mtg{gi}", tag=f"mtg{gi}"
                )
                nc.sync.dma_start(
                    out=mtg[:], in_=mtp_d[:, p0 * 128 : (p0 + ng) * 128]
                )
                mng = constp.tile(
                    [128, ng * 2 * DP1], bf16, name=f"mng{gi}", tag=f"mng{gi}"
                )
                # mn transfers go out on the idle GPSIMD engine's DMA queue so
                # they stream in parallel with the sync-queue mt/xt transfers
                # (single-queue input DMA measured only ~47 GB/s, finishing
                # at 83us -- marginal against mid-sweep weight consumption)
                nc.gpsimd.dma_start(
                    out=mng[:], in_=mnp_d[:, p0 * 2 * DP1 : (p0 + ng) * 2 * DP1]
                )
                mt_g.append(mtg)
                mn_g.append(mng)
                p0 += ng
                if gi == 0:
                    for c in range(NBT):
                        nc.sync.dma_start(
                            out=xtd[0:D, c * BT : (c + 1) * BT],
                            in_=xt_d[:, c * BT : (c + 1) * BT],
                        )
                        nc.sync.dma_start(
                            out=xtd[64 : 64 + D, c * BT : (c + 1) * BT],
                            in_=xt_d[:, c * BT : (c + 1) * BT],
                        )

            accs = [
                acc_pool.tile([128, BT], f32, name=f"acc{b}", tag=f"acc{b}")
                for b in range(NBT_LIVE)
            ]
            out_sb = constp.tile([128, B], bf16)

            # HAM warm-up: the PE clock gate only opens after a ~3.4us window
            # of uninterrupted busy; the steady state stalls briefly every
            # unit, so without this the whole kernel can run at 1.2 GHz.
            # These dummy matmuls run back-to-back (no cross-engine deps)
            # while the input DMAs stream, costing no wall time.
            scratch = constp.tile([128, BT], fp16)
            nc.vector.memset(scratch[:], 0.0)
            for w in range(NWARM):
                stw = st_pool.tile([128, 2 * BT], f32, name="st", tag="st")
                nc.tensor.matmul(
                    stw[:, 0:BT],
                    scratch[:, 0:128],
                    scratch[:, 0:BT],
                    start=True,
                    stop=True,
                )

            def flush(ent):
                p, lb, mov_e, mov_o = ent
                gi, gp0 = _group_of_pair(p)
                mng = mn_g[gi]
                lp = p - gp0
                nc.tensor.matmul(
                    accs[lb][0:DP1, :],
                    mng[:, lp * 2 * DP1 : lp * 2 * DP1 + DP1],
                    mov_e,
                    start=(p == 0),
                    stop=(p == NPAIR - 1),
                    tile_position=(0, 0),
                )
                nc.tensor.matmul(
                    accs[lb][64 : 64 + DP1, :],
                    mng[:, lp * 2 * DP1 + DP1 : (lp + 1) * 2 * DP1],
                    mov_o,
                    start=(p == 0),
                    stop=(p == NPAIR - 1),
                    tile_position=(0, 64),
                )

            def copy_out(sweep):
                # copy the previous sweep's accumulators to SBUF + DMA out.
                # Deferred into the next sweep so the PE pipeline never
                # drains at the boundary (a >1us PE idle gap re-throttles
                # the HAM clock gate to 1.2 GHz for the rest of the kernel).
                # split the two copies across ACT and DVE so they run
                # concurrently (DVE is idle at the tail), and issue each DMA
                # from its copy's engine -> two queues transfer in parallel
                for lb in range(NBT_LIVE):
                    bt = sweep * NBT_LIVE + lb
                    dst = out_sb[:, bt * BT : (bt + 1) * BT]
                    if lb == 0:
                        nc.scalar.copy(dst, accs[lb][:])
                        dma_eng = nc.sync
                    else:
                        nc.vector.tensor_copy(dst, accs[lb][:])
                        dma_eng = nc.scalar
                    dma_eng.dma_start(
                        out=out_d[0:115, bt * BT : (bt + 1) * BT],
                        in_=out_sb[0:115, bt * BT : (bt + 1) * BT],
                    )

            u = 0
            for sweep in range(NBT // NBT_LIVE):
                pending = []
                for p in range(NPAIR):
                    gi, gp0 = _group_of_pair(p)
                    mtg = mt_g[gi]
                    lp = p - gp0
                    if sweep > 0 and p == 2:
                        copy_out(sweep - 1)
                    for lb in range(NBT_LIVE):
                        bt = sweep * NBT_LIVE + lb
                        st = st_pool.tile([128, 2 * BT], f32)
                        nc.tensor.matmul(
                            st[:, 0:BT],
                            mtg[0:D, lp * 128 : (lp + 1) * 128],
                            xtd[0:D, bt * BT : (bt + 1) * BT],
                            start=True,
                            stop=True,
                            tile_position=(0, 0),
                        )
                        nc.tensor.matmul(
                            st[:, BT : 2 * BT],
                            mtg[64 : 64 + D, lp * 128 : (lp + 1) * 128],
                            xtd[64 : 64 + D, bt * BT : (bt + 1) * BT],
                            start=True,
                            stop=True,
                            tile_position=(64, 0),
                        )
                        pt = pt_pool.tile([128, 2 * BT], f32)
                        ptb = pt[:].bitcast(bf16)  # [128, 4*BT]
                        # strict 1:1 alternation: each 2-unit block gets one
                        # ACT and one DVE exp. 21:19 (throughput-balanced)
                        # ties within noise; 1:1 holds the best verified
                        # full-clock measurement (155.7us).
                        if u % DVE_PERIOD == DVE_SLOT:
                            nc.vector.tensor_scalar(pt[:], st[:], C1, C2, mult, add)
                            mov_e = ptb[:, 0 : 2 * BT : 2]
                            mov_o = ptb[:, 2 * BT : 4 * BT : 2]
                        else:
                            nc.scalar.activation(
                                ptb[:, 0 : 2 * BT], st[:], Exp, bias=bias[:]
                            )
                            mov_e = ptb[:, 0:BT]
                            mov_o = ptb[:, BT : 2 * BT]
                        pending.append((p, lb, mov_e, mov_o))
                        # flush MM2s two units at a time on odd units, so the
                        # PE stream has runs of same-stationary-type matmuls
                        # (back-to-back same-type MMs fully hide the array
                        # drain; alternating types pays ~170ns per switch)
                        if u % 2 == 1:
                            while len(pending) > LAG:
                                flush(pending.pop(0))
                        u += 1
                for ent in pending:
                    flush(ent)
            copy_out(NBT // NBT_LIVE - 1)

    nc.compile()
    return nc


def _get_nc():
    global _nc_cache
    if _nc_cache is None:
        _nc_cache = _build()
    return _nc_cache


def _prep_inputs(x, M):
    x = np.asarray(x, dtype=np.float32)
    M = np.asarray(M, dtype=np.float32)

    xt = np.ascontiguousarray(x.T).astype(np.float16)  # [50, 2048]

    in_maps = []
    for i in range(N_CORES):
        Msh = np.zeros((KSHP, D), np.float32)
        Msh[:KSH] = M[i * KSH : (i + 1) * KSH]
        ch = Msh.reshape(NCHUNK, 128, D)
        mtp = np.zeros((128, NPAIR * 128), np.float16)
        mtp[0:D] = ch[0::2].transpose(2, 0, 1).reshape(D, NPAIR * 128)
        mtp[64 : 64 + D] = ch[1::2].transpose(2, 0, 1).reshape(D, NPAIR * 128)
        Mn = np.zeros((KSHP, DP1), np.float32)
        Mn[:KSH, :D] = Msh[:KSH]
        Mn[:KSH, D] = 1.0
        mnp = np.ascontiguousarray(
            Mn.reshape(NCHUNK, 128, DP1).transpose(1, 0, 2)
        ).reshape(128, NCHUNK * DP1).astype(ml_dtypes.bfloat16)
        in_maps.append({"xt": xt, "mtp": mtp, "mnp": mnp})
    return in_maps


def _run(x, M, trace=False):
    if trace:
        _install_trace_support()
    nc = _get_nc()
    in_maps = _prep_inputs(x, M)
    res = run_bass_kernel_spmd(nc, in_maps, core_ids=list(range(N_CORES)), trace=trace)
    x = np.asarray(x, dtype=np.float32)
    total = np.zeros((DP1, B), np.float64)
    for i in range(N_CORES):
        raw = res.results[i]["outU"]  # [128, 2048]
        total += raw[0:DP1].astype(np.float64)
        total += raw[64 : 64 + DP1].astype(np.float64)
    u = (total[:D] / total[D : D + 1]).T.astype(np.float32)
    out = np.concatenate([x, u], axis=1)
    return out, res


def kernel(x, M):
    out, _ = _run(x, M, trace=False)
    return out
